# revision 14
# baseline (speedup 1.0000x reference)
"""
Trainium2 Bass kernel for nn_AtomBondConsistencyLayer.

Math (see reference):
  atom_logits = silu(h@Wa1+ba1)@Wa2+ba2                         [N,119]
  bond_logits = silu(h[ii]@Wb1[:D] + h[jj]@Wb1[D:] + bb1)@Wb2+bb2  [P,5]
  valency    = silu(h@Wv1[:D] + softmax(atom_logits)@Wv1[D:]+bv1)@Wv2+bv2
  atom_pairs = triu pairs (i<j)                                  [P,2]

Key algebraic identity: h[ii]@W == (h@W)[ii], so the two huge [P,256]@[256,256]
GEMMs collapse to A=h@Wb1[:D], B=h@Wb1[D:] ([N,256] each, computed on host —
0.1% of FLOPs) plus the dominant per-pair work which runs on 8 NeuronCores:

  for each row-group i (pairs (i, j) with j=i+1..N-1):
      bond_hidden.T[:, j] = silu(B.T[:, j] + (A.T[:, i] + bb1))   <- ACT engine,
          bias folded into the activation's per-partition bias operand
      bond_logits.T = Wb2.T @ bond_hidden.T                        <- PE, bf16
  (+ bb2 added on host during assembly)

Sharding: group i -> core (i mod 8), so every core gets ~P/8 pairs AND ~128
groups (balanced instruction count). SPMD requires one identical graph on all
cores, so per-core inputs are pre-shifted slabs: slab_c[:, col] = B.T[:, col+c]
and slot widths padded to the core-0 width; junk columns are discarded on host.
"""

import sys
import numpy as np

sys.path.insert(0, "/opt/trn_rl_repo")

N = 1024
D = 256
NCORES = 8
NSLOT = 128  # slots per core; slot s on core c handles group i = 8*s + c
FP = [N - 1 - 8 * s for s in range(NSLOT)]  # padded slot widths (core-0 width)
OFFS = np.concatenate([[0], np.cumsum(FP)]).astype(np.int64)
TOT = int(OFFS[-1])  # 65920 padded pairs per core
P = N * (N - 1) // 2  # 523776

_NC_CACHE = {}

# matmul chunks of <=512 free columns, in slot order
CHUNKS = []
for _s in range(NSLOT):
    _c0 = 0
    while _c0 < FP[_s]:
        CHUNKS.append((_s, _c0, min(512, FP[_s] - _c0)))
        _c0 += 512
PACK = 3  # chunks per psum bank at offsets 0/32/64 (PE quadrant 3 unusable)
NPACKS = (len(CHUNKS) + PACK - 1) // PACK  # 64


def _build_nc():
    import concourse.bass as bass
    import concourse.bacc as bacc
    import concourse.mybir as mybir
    import concourse.tile as tile

    f32 = mybir.dt.float32
    bf16 = mybir.dt.bfloat16

    nc = bacc.Bacc(
        "TRN2", target_bir_lowering=False, debug=False, num_devices=NCORES
    )
    bT_ext = nc.declare_dram_parameter("bT", [2, 128, N], f32, isOutput=False)
    abT_ext = nc.declare_dram_parameter("abT", [2, 128, NSLOT], f32, isOutput=False)
    wb2_ext = nc.declare_dram_parameter("wb2", [2, 128, 5], f32, isOutput=False)
    out_ext = nc.declare_dram_parameter("bondT", [69, NPACKS * 512], bf16, isOutput=True)

    with tile.TileContext(nc) as tc:
        with (
            tc.tile_pool(name="weights", bufs=1) as wpool,
            tc.tile_pool(name="sil", bufs=3) as spool,
            tc.tile_pool(name="psum", bufs=3, space=bass.MemorySpace.PSUM) as ppool,
        ):
            bT = wpool.tile([128, 2, N], f32)
            abT = wpool.tile([128, 2, NSLOT], f32)
            wb2f = wpool.tile([128, 2, 5], f32)
            wb2 = wpool.tile([128, 2, 5], bf16)
            big_ob = wpool.tile([69, NPACKS * 512], bf16)
            for d in (0, 1):
                nc.sync.dma_start(bT[:, d, :], bT_ext[d])
                nc.sync.dma_start(abT[:, d, :], abT_ext[d])
                nc.sync.dma_start(wb2f[:, d, :], wb2_ext[d])
            nc.vector.tensor_copy(wb2[:], wb2f[:])

            sil_tiles = {}
            ps = None
            for idx, (s, c0, ck) in enumerate(CHUNKS):
                t, g = idx // PACK, idx % PACK
                if g == 0:
                    ps = ppool.tile([128, 512], f32, tag="ps")
                if c0 == 0:
                    # first chunk of slot s: run the two silu halves
                    Fp = FP[s]
                    j0 = 8 * s + 1
                    sil = spool.tile([128, 2, FP[0]], bf16, tag="sil")
                    sil_tiles[s] = sil
                    for d in (0, 1):
                        nc.scalar.activation(
                            sil[:, d, :Fp],
                            bT[:, d, j0 : j0 + Fp],
                            mybir.ActivationFunctionType.Silu,
                            bias=abT[:, d, s : s + 1],
                        )
                sil = sil_tiles[s]
                for d in (0, 1):
                    nc.tensor.matmul(
                        ps[32 * g : 32 * g + 5, :ck],
                        wb2[:, d, :],
                        sil[:, d, c0 : c0 + ck],
                        start=(d == 0),
                        stop=(d == 1),
                    )
                if g == PACK - 1 or idx == len(CHUNKS) - 1:
                    nrow = 32 * g + 5
                    nc.vector.tensor_copy(
                        big_ob[:nrow, t * 512 : (t + 1) * 512], ps[:nrow, :]
                    )
                    nc.sync.dma_start(
                        out_ext[:nrow, t * 512 : (t + 1) * 512],
                        big_ob[:nrow, t * 512 : (t + 1) * 512],
                    )
    nc.compile()
    return nc


def _get_nc():
    if "nc" not in _NC_CACHE:
        _NC_CACHE["nc"] = _build_nc()
    return _NC_CACHE["nc"]


def _silu(x):
    with np.errstate(over="ignore"):
        return (x / (1.0 + np.exp(-x))).astype(np.float32)


def _f32(x):
    return np.ascontiguousarray(np.asarray(x), dtype=np.float32)


def _kernel_impl(inputs, trace=False):
    from concourse.bass_utils import run_bass_kernel_spmd

    h = _f32(inputs["h"])
    Wa1, ba1 = _f32(inputs["Wa1"]), _f32(inputs["ba1"])
    Wa2, ba2 = _f32(inputs["Wa2"]), _f32(inputs["ba2"])
    Wb1, bb1 = _f32(inputs["Wb1"]), _f32(inputs["bb1"])
    Wb2, bb2 = _f32(inputs["Wb2"]), _f32(inputs["bb2"])
    Wv1, bv1 = _f32(inputs["Wv1"]), _f32(inputs["bv1"])
    Wv2, bv2 = _f32(inputs["Wv2"]), _f32(inputs["bv2"])

    # --- tiny host-side precompute (0.2% of total FLOPs) ---
    A = h @ Wb1[:D]            # [N, 256]
    B = h @ Wb1[D:]            # [N, 256]
    ABb = A + bb1[None, :]     # bias folded: silu(B[j] + ABb[i]) per pair
    BT = np.zeros((D, N + 7), dtype=np.float32)
    BT[:, :N] = B.T
    ABbT = ABb.T               # [256, N]

    in_maps = []
    for c in range(NCORES):
        slab = BT[:, c : c + N]                   # pre-shifted by core id
        cols = (8 * np.arange(NSLOT) + c).clip(max=N - 1)
        abT_c = ABbT[:, cols]                     # [256, 128]
        in_maps.append(
            {
                "bT": np.ascontiguousarray(slab.reshape(2, 128, N)),
                "abT": np.ascontiguousarray(abT_c.reshape(2, 128, NSLOT)),
                "wb2": np.ascontiguousarray(Wb2.reshape(2, 128, 5)),
            }
        )

    nc = _get_nc()
    res = run_bass_kernel_spmd(nc, in_maps, core_ids=list(range(NCORES)), trace=trace)

    # --- assemble bond_logits [P, 5] from the packed [125, NPACKS*512] layout ---
    bond = np.empty((P, 5), dtype=np.float32)
    for c in range(NCORES):
        slab = np.asarray(res.results[c]["bondT"], dtype=np.float32)
        for idx, (s, c0, ck) in enumerate(CHUNKS):
            i = 8 * s + c
            if i > N - 2:
                continue
            F = N - 1 - i  # valid width of this slot on core c
            v = min(ck, F - c0)  # valid width of this chunk
            if v <= 0:
                continue
            t, g = idx // PACK, idx % PACK
            goff = i * (N - 1) - i * (i - 1) // 2
            bond[goff + c0 : goff + c0 + v] = slab[
                32 * g : 32 * g + 5, 512 * t : 512 * t + v
            ].T
    bond += bb2[None, :]

    # --- tiny host-side MLPs (atom, valency) ---
    atom_logits = _silu(h @ Wa1 + ba1) @ Wa2 + ba2          # [N, 119]
    m = atom_logits.max(axis=-1, keepdims=True)
    e = np.exp(atom_logits - m)
    probs = (e / e.sum(axis=-1, keepdims=True)).astype(np.float32)
    vh = _silu(h @ Wv1[:D] + probs @ Wv1[D:] + bv1)
    valency = vh @ Wv2 + bv2                                 # [N, 1]

    ii, jj = np.triu_indices(N, k=1)
    atom_pairs = np.stack([ii, jj], axis=1).astype(np.int32)

    outs = (
        atom_logits.astype(np.float32),
        bond.astype(np.float32),
        valency.astype(np.float32),
        atom_pairs,
    )
    return outs, res


def kernel(**inputs):
    outs, _ = _kernel_impl(inputs, trace=False)
    return outs


# revision 17
# speedup vs baseline: 1.1917x; 1.1917x over previous
"""
Trainium2 Bass kernel for nn_AtomBondConsistencyLayer.

Math (see reference):
  atom_logits = silu(h@Wa1+ba1)@Wa2+ba2                         [N,119]
  bond_logits = silu(h[ii]@Wb1[:D] + h[jj]@Wb1[D:] + bb1)@Wb2+bb2  [P,5]
  valency    = silu(h@Wv1[:D] + softmax(atom_logits)@Wv1[D:]+bv1)@Wv2+bv2
  atom_pairs = triu pairs (i<j)                                  [P,2]

Key algebraic identity: h[ii]@W == (h@W)[ii], so the two huge [P,256]@[256,256]
GEMMs collapse to A=h@Wb1[:D], B=h@Wb1[D:] ([N,256] each, computed on host —
0.1% of FLOPs) plus the dominant per-pair work which runs on 8 NeuronCores:

  for each row-group i (pairs (i, j) with j=i+1..N-1):
      bond_hidden.T[:, j] = silu(B.T[:, j] + (A.T[:, i] + bb1))   <- ACT engine,
          bias folded into the activation's per-partition bias operand
      bond_logits.T = Wb2.T @ bond_hidden.T                        <- PE, bf16
  (+ bb2 added on host during assembly)

Sharding: group i -> core (i mod 8), so every core gets ~P/8 pairs AND ~128
groups (balanced instruction count). SPMD requires one identical graph on all
cores, so per-core inputs are pre-shifted slabs: slab_c[:, col] = B.T[:, col+c]
and slot widths padded to the core-0 width; junk columns are discarded on host.
"""

import sys
import numpy as np

sys.path.insert(0, "/opt/trn_rl_repo")

N = 1024
D = 256
NCORES = 8
NSLOT = 128  # slots per core; slot s on core c handles group i = 8*s + c
FP = [N - 1 - 8 * s for s in range(NSLOT)]  # padded slot widths (core-0 width)
OFFS = np.concatenate([[0], np.cumsum(FP)]).astype(np.int64)
TOT = int(OFFS[-1])  # 65920 padded pairs per core
P = N * (N - 1) // 2  # 523776

_NC_CACHE = {}

# matmul chunks of <=512 free columns, in slot order
CHUNKS = []
for _s in range(NSLOT):
    _c0 = 0
    while _c0 < FP[_s]:
        CHUNKS.append((_s, _c0, min(512, FP[_s] - _c0)))
        _c0 += 512
N_OUT_CHUNKS = 16  # output DMA granularity (slot-aligned)


def _build_nc():
    import concourse.bass as bass
    import concourse.bacc as bacc
    import concourse.mybir as mybir
    import concourse.tile as tile

    f32 = mybir.dt.float32
    bf16 = mybir.dt.bfloat16

    nc = bacc.Bacc(
        "TRN2", target_bir_lowering=False, debug=False, num_devices=NCORES
    )
    bT_ext = nc.declare_dram_parameter("bT", [2, 128, N], f32, isOutput=False)
    abT_ext = nc.declare_dram_parameter("abT", [2, 128, NSLOT], f32, isOutput=False)
    wb2_ext = nc.declare_dram_parameter("wb2", [2, 128, 5], f32, isOutput=False)
    out_ext = nc.declare_dram_parameter("bondT", [5, TOT], bf16, isOutput=True)

    with tile.TileContext(nc) as tc:
        with (
            tc.tile_pool(name="weights", bufs=1) as wpool,
            tc.tile_pool(name="sil", bufs=3) as spool,
            tc.tile_pool(name="psum", bufs=4, space=bass.MemorySpace.PSUM) as ppool,
        ):
            bT = wpool.tile([128, 2, N], f32)
            abT = wpool.tile([128, 2, NSLOT], f32)
            wb2f = wpool.tile([128, 2, 5], f32)
            wb2 = wpool.tile([128, 2, 5], bf16)
            big_ob = wpool.tile([5, TOT], bf16)
            for d in (0, 1):
                nc.sync.dma_start(bT[:, d, :], bT_ext[d])
                nc.sync.dma_start(abT[:, d, :], abT_ext[d])
                nc.sync.dma_start(wb2f[:, d, :], wb2_ext[d])
            nc.vector.tensor_copy(wb2[:], wb2f[:])

            # output DMA chunk boundaries, aligned to slot offsets
            slot_bounds = [0]
            for t in range(1, N_OUT_CHUNKS):
                target = TOT * t // N_OUT_CHUNKS
                slot_bounds.append(min(int(np.searchsorted(OFFS, target)), NSLOT))
            slot_bounds.append(NSLOT)

            for s in range(NSLOT):
                Fp = FP[s]
                off = int(OFFS[s])
                j0 = 8 * s + 1
                sil = spool.tile([128, 2, FP[0]], bf16, tag="sil")
                for d in (0, 1):
                    nc.scalar.activation(
                        sil[:, d, :Fp],
                        bT[:, d, j0 : j0 + Fp],
                        mybir.ActivationFunctionType.Silu,
                        bias=abT[:, d, s : s + 1],
                    )
                c0 = 0
                while c0 < Fp:
                    ck = min(512, Fp - c0)
                    ps = ppool.tile([5, 512], f32, tag="ps")
                    for d in (0, 1):
                        nc.tensor.matmul(
                            ps[:, :ck],
                            wb2[:, d, :],
                            sil[:, d, c0 : c0 + ck],
                            start=(d == 0),
                            stop=(d == 1),
                        )
                    nc.vector.tensor_copy(
                        big_ob[:, off + c0 : off + c0 + ck], ps[:, :ck]
                    )
                    c0 += ck
                for t in range(1, N_OUT_CHUNKS + 1):
                    if slot_bounds[t] == s + 1 and slot_bounds[t] > slot_bounds[t - 1]:
                        a = int(OFFS[slot_bounds[t - 1]])
                        b = int(OFFS[slot_bounds[t]])
                        if b > a:
                            nc.sync.dma_start(out_ext[:, a:b], big_ob[:, a:b])
    nc.compile()
    return nc


def _get_nc():
    if "nc" not in _NC_CACHE:
        _NC_CACHE["nc"] = _build_nc()
    return _NC_CACHE["nc"]


def _silu(x):
    with np.errstate(over="ignore"):
        return (x / (1.0 + np.exp(-x))).astype(np.float32)


def _f32(x):
    return np.ascontiguousarray(np.asarray(x), dtype=np.float32)


def _kernel_impl(inputs, trace=False):
    from concourse.bass_utils import run_bass_kernel_spmd

    h = _f32(inputs["h"])
    Wa1, ba1 = _f32(inputs["Wa1"]), _f32(inputs["ba1"])
    Wa2, ba2 = _f32(inputs["Wa2"]), _f32(inputs["ba2"])
    Wb1, bb1 = _f32(inputs["Wb1"]), _f32(inputs["bb1"])
    Wb2, bb2 = _f32(inputs["Wb2"]), _f32(inputs["bb2"])
    Wv1, bv1 = _f32(inputs["Wv1"]), _f32(inputs["bv1"])
    Wv2, bv2 = _f32(inputs["Wv2"]), _f32(inputs["bv2"])

    # --- tiny host-side precompute (0.2% of total FLOPs) ---
    A = h @ Wb1[:D]            # [N, 256]
    B = h @ Wb1[D:]            # [N, 256]
    ABb = A + bb1[None, :]     # bias folded: silu(B[j] + ABb[i]) per pair
    BT = np.zeros((D, N + 7), dtype=np.float32)
    BT[:, :N] = B.T
    ABbT = ABb.T               # [256, N]

    in_maps = []
    for c in range(NCORES):
        slab = BT[:, c : c + N]                   # pre-shifted by core id
        cols = (8 * np.arange(NSLOT) + c).clip(max=N - 1)
        abT_c = ABbT[:, cols]                     # [256, 128]
        in_maps.append(
            {
                "bT": np.ascontiguousarray(slab.reshape(2, 128, N)),
                "abT": np.ascontiguousarray(abT_c.reshape(2, 128, NSLOT)),
                "wb2": np.ascontiguousarray(Wb2.reshape(2, 128, 5)),
            }
        )

    nc = _get_nc()
    res = run_bass_kernel_spmd(nc, in_maps, core_ids=list(range(NCORES)), trace=trace)

    # --- assemble bond_logits [P, 5] ---
    bond = np.empty((P, 5), dtype=np.float32)
    for c in range(NCORES):
        slabT = np.asarray(res.results[c]["bondT"], dtype=np.float32)  # [5, TOT]
        for s in range(NSLOT):
            i = 8 * s + c
            if i > N - 2:
                continue
            F = N - 1 - i
            goff = i * (N - 1) - i * (i - 1) // 2
            q0 = int(OFFS[s])
            bond[goff : goff + F] = slabT[:, q0 : q0 + F].T
    bond += bb2[None, :]

    # --- tiny host-side MLPs (atom, valency) ---
    atom_logits = _silu(h @ Wa1 + ba1) @ Wa2 + ba2          # [N, 119]
    m = atom_logits.max(axis=-1, keepdims=True)
    e = np.exp(atom_logits - m)
    probs = (e / e.sum(axis=-1, keepdims=True)).astype(np.float32)
    vh = _silu(h @ Wv1[:D] + probs @ Wv1[D:] + bv1)
    valency = vh @ Wv2 + bv2                                 # [N, 1]

    ii, jj = np.triu_indices(N, k=1)
    atom_pairs = np.stack([ii, jj], axis=1).astype(np.int32)

    outs = (
        atom_logits.astype(np.float32),
        bond.astype(np.float32),
        valency.astype(np.float32),
        atom_pairs,
    )
    return outs, res


def kernel(**inputs):
    outs, _ = _kernel_impl(inputs, trace=False)
    return outs
